# revision 6
# baseline (speedup 1.0000x reference)
"""AxialAttention Trainium2 kernel: 8-core SPMD, no collectives.

Sharding: core (b, j) computes height-attention for x[b, :, 64j:64j+64, :]
and width-attention for x[b, 32j:32j+32, :, :]; host sums partial outputs.

Bisection variant: baseline scores/exp/AV/PSUM structure + new vaug/pb big
tiles, sync-queue normalize DMAs, and q-copies on ScalarE.
"""

import numpy as np
import ml_dtypes

B, H, W, C = 2, 128, 256, 256
HEADS, D = 8, 32
SCALE = float(D) ** -0.5
WC = W // 4   # 64 w-columns per core (height phase)
HC = H // 4   # 32 h-rows per core (width phase)
NTOK = 8192   # tokens per core per phase
BF16 = ml_dtypes.bfloat16

_compiled = {}


def _build_module():
    import contextlib
    import concourse.bass as bass  # noqa: F401
    from concourse import bacc, mybir
    from concourse.tile import TileContext

    bf = mybir.dt.bfloat16
    f32 = mybir.dt.float32
    Exp = mybir.ActivationFunctionType.Exp
    mult = mybir.AluOpType.mult

    nc = bacc.Bacc("TRN2", target_bir_lowering=False)

    # ---- DRAM I/O ----
    xh = nc.dram_tensor("xh", [2, 128, NTOK], bf, kind="ExternalInput")
    xw = nc.dram_tensor("xw", [2, 128, NTOK], bf, kind="ExternalInput")
    wts = {}
    for ph in ("h", "w"):
        wts[f"wq_{ph}"] = nc.dram_tensor(f"wq_{ph}", [2, 128, 256], bf, kind="ExternalInput")
        wts[f"wk_{ph}"] = nc.dram_tensor(f"wk_{ph}", [2, 128, 256], bf, kind="ExternalInput")
        wts[f"wv_{ph}"] = nc.dram_tensor(f"wv_{ph}", [2, 128, 256], bf, kind="ExternalInput")
        wts[f"wo_{ph}"] = nc.dram_tensor(f"wo_{ph}", [4, 2, 128, 128], bf, kind="ExternalInput")
    out_h = nc.dram_tensor("out_h", [2, 128, WC * 128], f32, kind="ExternalOutput")
    out_w = nc.dram_tensor("out_w", [2, 128, HC * 256], f32, kind="ExternalOutput")

    def phase(tc, ctx, xT_dram, wq_d, wk_d, wv_d, wo_d, out_d, is_width):
        tag = "w" if is_width else "h"
        XBLK = 256 if is_width else 128          # attention span per block
        nblk = HC if is_width else WC            # 32 or 64 blocks
        half = nblk // 2
        AVW = 4 * XBLK                           # av psum width: 4 pairs
        NT = NTOK // 128                         # 64 token tiles
        GRP = 2 if is_width else 4               # blocks per qk projection group
        OG = 2 if is_width else 4                # blocks per oproj group

        pool = ctx.enter_context(tc.tile_pool(name="persist", bufs=1))
        work = ctx.enter_context(tc.tile_pool(name="work", bufs=2))
        at_pool = ctx.enter_context(tc.tile_pool(name="at", bufs=3 if not is_width else 2))
        bc_pool = ctx.enter_context(tc.tile_pool(name="bc", bufs=3))
        pbn_pool = ctx.enter_context(tc.tile_pool(name="pbn", bufs=2))
        osb_pool = ctx.enter_context(tc.tile_pool(name="osb", bufs=2))
        ps_s = ctx.enter_context(tc.tile_pool(name="ps_s", bufs=1, space="PSUM"))
        ps_av = ctx.enter_context(tc.tile_pool(name="ps_av", bufs=2 if not is_width else 1, space="PSUM"))
        ps_m = ctx.enter_context(tc.tile_pool(name="ps_m", bufs=2, space="PSUM"))

        # ---- weights + xT to SBUF ----
        def load(dram_ap, shape, nm):
            t = pool.tile(shape, bf, tag=nm, name=nm)
            nc.sync.dma_start(t[:], dram_ap)
            return t

        wq_sb = [load(wq_d[cc], [128, 256], f"wq{cc}") for cc in range(2)]
        wk_sb = [load(wk_d[cc], [128, 256], f"wk{cc}") for cc in range(2)]
        wv_sb = [load(wv_d[cc], [128, 256], f"wv{cc}") for cc in range(2)]
        wo_sb = [[load(wo_d[p, cc], [128, 128], f"wo{p}_{cc}") for cc in range(2)]
                 for p in range(4)]
        xT = [load(xT_dram[cc], [128, NTOK], f"xT{cc}") for cc in range(2)]

        # ---- v_aug big tile: per head [v_h | 1 | 0...] in 64-col slots ----
        vaug = pool.tile([128, NT * 512], bf, tag="vaug", name="vaug")
        nc.vector.memset(vaug[:], 0.0)
        nc.vector.memset(
            vaug[:].rearrange("p (t h c) -> p t h c", h=8, c=64)[:, :, :, 32], 1.0)
        for t_i in range(NT):
            ps = ps_m.tile([128, 512], f32, tag="ps_m", name="ps_m")
            for cc in range(2):
                nc.tensor.matmul(
                    ps[:, 0:256], xT[cc][:, t_i * 128:(t_i + 1) * 128], wv_sb[cc][:],
                    start=(cc == 0), stop=(cc == 1))
            nc.vector.tensor_copy(
                vaug[:, t_i * 512:(t_i + 1) * 512]
                    .rearrange("p (h c) -> p h c", h=8, c=64)[:, :, 0:32],
                ps[:, 0:256].rearrange("p (h c) -> p h c", h=8, c=32))

        # ---- attention main loops ----
        dn = pool.tile([2 * nblk, AVW], bf, tag="dn", name="dn")     # compacted denominators
        rec = pool.tile([2 * nblk, AVW], bf, tag="rec", name="rec")  # their reciprocals
        rec_d = nc.dram_tensor(f"rec_dram_{tag}", [2 * nblk, AVW], bf)
        pb = pool.tile([128, half * AVW], bf, tag="pb", name="pb")   # unnormalized stash

        for hf in range(2):
            blks = list(range(hf * half, (hf + 1) * half))
            qk_groups = {}
            for blk in blks:
                # --- grouped q/k projection over GRP blocks (512 tokens) ---
                g = blk // GRP
                if g not in qk_groups:
                    gtiles = []
                    for ti, w_sb in ((0, wq_sb), (1, wk_sb)):
                        gt = work.tile([128, 1024], bf, tag=f"qkg{ti}", name=f"qkg{ti}")
                        for ic in range(2):
                            psqk = ps_m.tile([128, 512], f32, tag="ps_m", name="ps_m")
                            for cc in range(2):
                                nc.tensor.matmul(
                                    psqk[:],
                                    w_sb[cc][:, ic * 128:(ic + 1) * 128],
                                    xT[cc][:, g * 512:(g + 1) * 512],
                                    start=(cc == 0), stop=(cc == 1))
                            if ti == 0:
                                nc.scalar.copy(gt[:, ic * 512:(ic + 1) * 512], psqk[:])
                            else:
                                nc.vector.tensor_copy(gt[:, ic * 512:(ic + 1) * 512], psqk[:])
                        gtiles.append(gt)
                    qk_groups = {g: gtiles}      # keep only current group
                qg, kg = qk_groups[g]
                boff = (blk % GRP) * 128 if not is_width else (blk % GRP) * 256

                if not is_width:
                    aT = at_pool.tile([128, 8 * 128], bf, tag="aT", name="aT")
                    ps = ps_s.tile([128, 2048], f32, tag="s_ps", name="s_ps")
                    for h in range(8):
                        th, hh = divmod(h, 4)
                        col = 512 * hh + 128 * th        # bank = row-group
                        nc.tensor.matmul(
                            ps[:, col:col + 128],
                            kg[hh * 32:(hh + 1) * 32, th * 512 + boff: th * 512 + boff + 128],
                            qg[hh * 32:(hh + 1) * 32, th * 512 + boff: th * 512 + boff + 128],
                            start=True, stop=True,
                            tile_position=(hh * 32, 0))
                    # aT col for head h=4*th+hh is 128*h = 512*th + 128*hh
                    nc.scalar.activation(
                        aT[:].rearrange("p (th hh x) -> p hh th x", th=2, hh=4),
                        ps[:].rearrange("p (hh b) -> p hh b", hh=4)[:, :, 0:256]
                             .rearrange("p hh (th x) -> p hh th x", th=2),
                        Exp, scale=SCALE)
                else:
                    aT = at_pool.tile([128, 2 * 8 * 256], bf, tag="aT", name="aT")
                    for yc in range(2):
                        ps = ps_s.tile([128, 2048], f32, tag="s_ps", name="s_ps")
                        for h in range(8):
                            th, hh = divmod(h, 4)
                            col = 512 * hh + 256 * th    # bank = row-group
                            nc.tensor.matmul(
                                ps[:, col:col + 256],
                                kg[hh * 32:(hh + 1) * 32, th * 512 + boff + yc * 128: th * 512 + boff + (yc + 1) * 128],
                                qg[hh * 32:(hh + 1) * 32, th * 512 + boff: th * 512 + boff + 256],
                                start=True, stop=True,
                                tile_position=(hh * 32, 0))
                        nc.scalar.activation(
                            aT[:, yc * 2048:(yc + 1) * 2048].rearrange(
                                "p (th hh x) -> p hh th x", th=2, hh=4),
                            ps[:].rearrange("p (hh b) -> p hh b", hh=4)[:, :, 0:512]
                                 .rearrange("p hh (th x) -> p hh th x", th=2),
                            Exp, scale=SCALE)

                # --- AV with denominator column, 2-head col packing per pair ---
                av = ps_av.tile([128, AVW], f32, tag="av_ps", name="av_ps")
                for p in range(4):
                    osl = slice(p * XBLK, (p + 1) * XBLK)
                    for s in range(2):
                        h = 2 * p + s
                        op = 64 * s
                        if not is_width:
                            nc.tensor.matmul(
                                av[op:op + 64, osl],
                                vaug[:, blk * 512 + h * 64: blk * 512 + (h + 1) * 64],
                                aT[:, h * 128:(h + 1) * 128],
                                start=True, stop=True,
                                tile_position=(0, op))
                        else:
                            for yc in range(2):
                                nc.tensor.matmul(
                                    av[op:op + 64, osl],
                                    vaug[:, (2 * blk + yc) * 512 + h * 64:
                                         (2 * blk + yc) * 512 + (h + 1) * 64],
                                    aT[:, yc * 2048 + h * 256:yc * 2048 + (h + 1) * 256],
                                    start=(yc == 0), stop=(yc == 1),
                                    tile_position=(0, op))

                # --- stash unnormalized block; compact denom rows via sync DMA ---
                bo = (blk % half) * AVW
                nc.vector.tensor_copy(pb[:, bo:bo + AVW], av[:])
                r0 = 2 * blk
                nc.sync.dma_start(dn[r0:r0 + 1, :], pb[32:33, bo:bo + AVW])
                nc.sync.dma_start(dn[r0 + 1:r0 + 2, :], pb[96:97, bo:bo + AVW])

            # --- reciprocal for this half; bounce to DRAM for partition bcast ---
            rsl = slice(hf * 2 * half, (hf + 1) * 2 * half)
            with nc.allow_low_precision(reason="bf16 softmax denominators"):
                nc.vector.reciprocal(rec[rsl, :], dn[rsl, :])
            nc.sync.dma_start(rec_d[rsl, :], rec[rsl, :])

            # --- normalize into OG-block pbn buffer + batched output projection ---
            pbn = None
            for blk in blks:
                r0 = 2 * blk
                bo = (blk % half) * AVW
                bc = bc_pool.tile([128, AVW], bf, tag="bc", name="bc")
                nc.sync.dma_start(
                    bc[0:64, :],
                    rec_d[r0:r0 + 1, :].partition_broadcast(64).rearrange("p o f -> p (o f)"))
                nc.sync.dma_start(
                    bc[64:128, :],
                    rec_d[r0 + 1:r0 + 2, :].partition_broadcast(64).rearrange("p o f -> p (o f)"))
                gi = blk % OG
                if gi == 0:
                    pbn = pbn_pool.tile([128, 4 * OG * XBLK], bf, tag="pbn", name="pbn")
                # pbn layout: [128, (p:4) (gi:OG) (x:XBLK)] so pair p spans 512 tokens
                pv = pbn[:].rearrange("q (p g x) -> q p g x", p=4, g=OG)
                nc.gpsimd.tensor_tensor(
                    pv[:, :, gi, :],
                    pb[:, bo:bo + AVW].rearrange("q (p x) -> q p x", p=4),
                    bc[:].rearrange("q (p x) -> q p x", p=4), mult)
                if gi == OG - 1:
                    g0 = (blk // OG) * OG        # first block of group
                    for cc in range(2):
                        po = ps_m.tile([128, 512], f32, tag="ps_m", name="ps_m")
                        for p in range(4):
                            nc.tensor.matmul(
                                po[:],
                                wo_sb[p][cc][:],
                                pbn[:, p * 512:(p + 1) * 512],
                                start=(p == 0), stop=(p == 3))
                        osb = osb_pool.tile([128, 512], f32, tag="osb", name="osb")
                        nc.vector.tensor_copy(osb[:], po[:])
                        nc.sync.dma_start(
                            out_d[cc][:, g0 * XBLK:g0 * XBLK + 512], osb[:])

    with TileContext(nc) as tc:
        with contextlib.ExitStack() as c1:
            phase(tc, c1, xh, wts["wq_h"], wts["wk_h"], wts["wv_h"], wts["wo_h"],
                  out_h, is_width=False)
        with contextlib.ExitStack() as c2:
            phase(tc, c2, xw, wts["wq_w"], wts["wk_w"], wts["wv_w"], wts["wo_w"],
                  out_w, is_width=True)

    nc.compile()
    return nc


def _prep_weights(inp):
    """Host-side weight layouts, bf16."""
    def chunks(Wm):                      # [256, 256] -> [2, 128, 256] (lhsT chunks)
        return np.ascontiguousarray(Wm.reshape(2, 128, 256)).astype(BF16)

    def wo_aug(Wo):                      # -> [4 pairs, 2 cc, 128 K(padded), 128 M]
        out = np.zeros((4, 2, 128, 128), np.float32)
        for p in range(4):
            for cc in range(2):
                out[p, cc, 0:32, :] = Wo[64 * p:64 * p + 32, cc * 128:(cc + 1) * 128]
                out[p, cc, 64:96, :] = Wo[64 * p + 32:64 * p + 64, cc * 128:(cc + 1) * 128]
        return out.astype(BF16)

    d = {}
    for ph in ("h", "w"):
        d[f"wq_{ph}"] = chunks(np.asarray(inp[f"Wq_{ph}"], np.float32))
        d[f"wk_{ph}"] = chunks(np.asarray(inp[f"Wk_{ph}"], np.float32))
        d[f"wv_{ph}"] = chunks(np.asarray(inp[f"Wv_{ph}"], np.float32))
        d[f"wo_{ph}"] = wo_aug(np.asarray(inp[f"Wo_{ph}"], np.float32))
    return d


def kernel(x, Wq_h, Wk_h, Wv_h, Wo_h, bo_h, Wq_w, Wk_w, Wv_w, Wo_w, bo_w, h, w,
           _trace=False):
    from concourse.bass_utils import run_bass_kernel_spmd

    x = np.asarray(x, np.float32)
    xs = x.reshape(B, H, W, C)
    wd = _prep_weights(dict(Wq_h=Wq_h, Wk_h=Wk_h, Wv_h=Wv_h, Wo_h=Wo_h,
                            Wq_w=Wq_w, Wk_w=Wk_w, Wv_w=Wv_w, Wo_w=Wo_w))

    in_maps = []
    for core in range(8):
        b, j = divmod(core, 4)
        xh_a = xs[b][:, j * WC:(j + 1) * WC, :].transpose(2, 1, 0)   # [C, Wc, H]
        xw_a = xs[b][j * HC:(j + 1) * HC, :, :].transpose(2, 0, 1)   # [C, Hc, W]
        m = dict(wd)
        m["xh"] = np.ascontiguousarray(xh_a).reshape(2, 128, NTOK).astype(BF16)
        m["xw"] = np.ascontiguousarray(xw_a).reshape(2, 128, NTOK).astype(BF16)
        in_maps.append(m)

    if "nc" not in _compiled:
        _compiled["nc"] = _build_module()
    nc = _compiled["nc"]

    kw = {}
    if _trace:
        kw = dict(trace=True, trace_cores=[0])
    res = run_bass_kernel_spmd(nc, in_maps, core_ids=list(range(8)), **kw)
    _compiled["last_result"] = res

    out = np.zeros((B, H, W, C), np.float32)
    for core in range(8):
        b, j = divmod(core, 4)
        oh = np.asarray(res.results[core]["out_h"], dtype=np.float32)
        ow = np.asarray(res.results[core]["out_w"], dtype=np.float32)
        # outT[c, n], c = cc*128 + ci; height n = w*128 + r -> [r, w, c]
        oh_t = oh.reshape(256, WC, 128).transpose(2, 1, 0)
        out[b, :, j * WC:(j + 1) * WC, :] += oh_t
        # width n = r*256 + wcol -> [r, wcol, c]
        ow_t = ow.reshape(256, HC, 256).transpose(1, 2, 0)
        out[b, j * HC:(j + 1) * HC, :, :] += ow_t
    out += np.asarray(bo_h, np.float32) + np.asarray(bo_w, np.float32)
    return out.reshape(B, H * W, C)


# revision 9
# speedup vs baseline: 1.2370x; 1.2370x over previous
"""AxialAttention Trainium2 kernel: 8-core SPMD, no collectives.

Sharding: core (b, j) computes height-attention for x[b, :, 64j:64j+64, :]
and width-attention for x[b, 32j:32j+32, :, :]; host sums partial outputs.

Bisection variant: baseline scores/exp/AV/PSUM structure + new vaug/pb big
tiles, sync-queue normalize DMAs, and q-copies on ScalarE.
"""

import numpy as np
import ml_dtypes

B, H, W, C = 2, 128, 256, 256
HEADS, D = 8, 32
SCALE = float(D) ** -0.5
WC = W // 4   # 64 w-columns per core (height phase)
HC = H // 4   # 32 h-rows per core (width phase)
NTOK = 8192   # tokens per core per phase
BF16 = ml_dtypes.bfloat16

_compiled = {}


def _build_module():
    import contextlib
    import concourse.bass as bass  # noqa: F401
    from concourse import bacc, mybir
    from concourse.tile import TileContext

    bf = mybir.dt.bfloat16
    f32 = mybir.dt.float32
    Exp = mybir.ActivationFunctionType.Exp
    mult = mybir.AluOpType.mult

    nc = bacc.Bacc("TRN2", target_bir_lowering=False)

    # ---- DRAM I/O ----
    xh = nc.dram_tensor("xh", [2, 128, NTOK], bf, kind="ExternalInput")
    xw = nc.dram_tensor("xw", [2, 128, NTOK], bf, kind="ExternalInput")
    wts = {}
    for ph in ("h", "w"):
        wts[f"wq_{ph}"] = nc.dram_tensor(f"wq_{ph}", [2, 128, 256], bf, kind="ExternalInput")
        wts[f"wk_{ph}"] = nc.dram_tensor(f"wk_{ph}", [2, 128, 256], bf, kind="ExternalInput")
        wts[f"wv_{ph}"] = nc.dram_tensor(f"wv_{ph}", [2, 128, 512], bf, kind="ExternalInput")
        wts[f"wo_{ph}"] = nc.dram_tensor(f"wo_{ph}", [4, 2, 128, 128], bf, kind="ExternalInput")
    out_h = nc.dram_tensor("out_h", [2, 128, WC * 128], f32, kind="ExternalOutput")
    out_w = nc.dram_tensor("out_w", [2, 128, HC * 256], f32, kind="ExternalOutput")

    def phase(tc, ctx, xT_dram, wq_d, wk_d, wv_d, wo_d, out_d, is_width):
        tag = "w" if is_width else "h"
        XBLK = 256 if is_width else 128          # attention span per block
        nblk = HC if is_width else WC            # 32 or 64 blocks
        half = nblk // 2
        AVW = 4 * XBLK                           # av psum width: 4 pairs
        NT = NTOK // 128                         # 64 token tiles
        GRP = 2 if is_width else 4               # blocks per qk projection group
        OG = 2 if is_width else 4                # blocks per oproj group

        pool = ctx.enter_context(tc.tile_pool(name="persist", bufs=1))
        work = ctx.enter_context(tc.tile_pool(name="work", bufs=2))
        at_pool = ctx.enter_context(tc.tile_pool(name="at", bufs=3 if not is_width else 2))
        bc_pool = ctx.enter_context(tc.tile_pool(name="bc", bufs=3))
        pbn_pool = ctx.enter_context(tc.tile_pool(name="pbn", bufs=2))
        osb_pool = ctx.enter_context(tc.tile_pool(name="osb", bufs=2))
        ps_s = ctx.enter_context(tc.tile_pool(name="ps_s", bufs=1, space="PSUM"))
        ps_av = ctx.enter_context(tc.tile_pool(name="ps_av", bufs=2 if not is_width else 1, space="PSUM"))
        ps_m = ctx.enter_context(tc.tile_pool(name="ps_m", bufs=2, space="PSUM"))

        # ---- weights + xT to SBUF ----
        def load(dram_ap, shape, nm):
            t = pool.tile(shape, bf, tag=nm, name=nm)
            nc.scalar.dma_start(t[:], dram_ap)
            return t

        wq_sb = [load(wq_d[cc], [128, 256], f"wq{cc}") for cc in range(2)]
        wk_sb = [load(wk_d[cc], [128, 256], f"wk{cc}") for cc in range(2)]
        wv_sb = [load(wv_d[cc], [128, 512], f"wv{cc}") for cc in range(2)]
        wo_sb = [[load(wo_d[p, cc], [128, 128], f"wo{p}_{cc}") for cc in range(2)]
                 for p in range(4)]
        xT = [load(xT_dram[cc], [128, NTOK], f"xT{cc}") for cc in range(2)]

        # ---- v_aug big tile: per head [v_h | 1 | 0] (zeros from padded Wv) ----
        vaug = pool.tile([128, NT * 512], bf, tag="vaug", name="vaug")
        for t_i in range(NT):
            ps = ps_m.tile([128, 512], f32, tag="ps_m", name="ps_m")
            for cc in range(2):
                nc.tensor.matmul(
                    ps[:], xT[cc][:, t_i * 128:(t_i + 1) * 128], wv_sb[cc][:],
                    start=(cc == 0), stop=(cc == 1))
            vsl = vaug[:, t_i * 512:(t_i + 1) * 512]
            nc.vector.tensor_copy(vsl, ps[:])
            nc.vector.memset(vsl.rearrange("p (h t) -> p h t", t=64)[:, :, 32], 1.0)

        # ---- attention main loops ----
        # dn row = hf*64 + s*32 + (blk % half)  (32-aligned (hf,s) groups)
        dn = pool.tile([128, AVW], bf, tag="dn", name="dn")          # compacted denominators
        rec = pool.tile([128, AVW], bf, tag="rec", name="rec")       # their reciprocals
        rec_d = nc.dram_tensor(f"rec_dram_{tag}", [128, AVW], bf)
        pb = pool.tile([128, half * AVW], bf, tag="pb", name="pb")   # unnormalized stash

        for hf in range(2):
            blks = list(range(hf * half, (hf + 1) * half))
            qk_groups = {}
            for blk in blks:
                # --- grouped q/k projection over GRP blocks (512 tokens) ---
                g = blk // GRP
                if g not in qk_groups:
                    gtiles = []
                    for ti, w_sb in ((0, wq_sb), (1, wk_sb)):
                        gt = work.tile([128, 1024], bf, tag=f"qkg{ti}", name=f"qkg{ti}")
                        for ic in range(2):
                            psqk = ps_m.tile([128, 512], f32, tag="ps_m", name="ps_m")
                            for cc in range(2):
                                nc.tensor.matmul(
                                    psqk[:],
                                    w_sb[cc][:, ic * 128:(ic + 1) * 128],
                                    xT[cc][:, g * 512:(g + 1) * 512],
                                    start=(cc == 0), stop=(cc == 1))
                            nc.vector.tensor_copy(gt[:, ic * 512:(ic + 1) * 512], psqk[:])
                        gtiles.append(gt)
                    qk_groups = {g: gtiles}      # keep only current group
                qg, kg = qk_groups[g]
                boff = (blk % GRP) * 128 if not is_width else (blk % GRP) * 256

                if not is_width:
                    aT = at_pool.tile([128, 8 * 128], bf, tag="aT", name="aT")
                    ps = ps_s.tile([128, 2048], f32, tag="s_ps", name="s_ps")
                    for h in range(8):
                        th, hh = divmod(h, 4)
                        col = 512 * hh + 128 * th        # bank = row-group
                        nc.tensor.matmul(
                            ps[:, col:col + 128],
                            kg[hh * 32:(hh + 1) * 32, th * 512 + boff: th * 512 + boff + 128],
                            qg[hh * 32:(hh + 1) * 32, th * 512 + boff: th * 512 + boff + 128],
                            start=True, stop=True,
                            tile_position=(hh * 32, 0))
                    # aT col for head h=4*th+hh is 128*h = 512*th + 128*hh
                    nc.scalar.activation(
                        aT[:].rearrange("p (th hh x) -> p hh th x", th=2, hh=4),
                        ps[:].rearrange("p (hh b) -> p hh b", hh=4)[:, :, 0:256]
                             .rearrange("p hh (th x) -> p hh th x", th=2),
                        Exp, scale=SCALE)
                else:
                    aT = at_pool.tile([128, 2 * 8 * 256], bf, tag="aT", name="aT")
                    for yc in range(2):
                        ps = ps_s.tile([128, 2048], f32, tag="s_ps", name="s_ps")
                        for h in range(8):
                            th, hh = divmod(h, 4)
                            col = 512 * hh + 256 * th    # bank = row-group
                            nc.tensor.matmul(
                                ps[:, col:col + 256],
                                kg[hh * 32:(hh + 1) * 32, th * 512 + boff + yc * 128: th * 512 + boff + (yc + 1) * 128],
                                qg[hh * 32:(hh + 1) * 32, th * 512 + boff: th * 512 + boff + 256],
                                start=True, stop=True,
                                tile_position=(hh * 32, 0))
                        nc.scalar.activation(
                            aT[:, yc * 2048:(yc + 1) * 2048].rearrange(
                                "p (th hh x) -> p hh th x", th=2, hh=4),
                            ps[:].rearrange("p (hh b) -> p hh b", hh=4)[:, :, 0:512]
                                 .rearrange("p hh (th x) -> p hh th x", th=2),
                            Exp, scale=SCALE)

                # --- AV with denominator column, 2-head col packing per pair ---
                av = ps_av.tile([128, AVW], f32, tag="av_ps", name="av_ps")
                for p in range(4):
                    osl = slice(p * XBLK, (p + 1) * XBLK)
                    for s in range(2):
                        h = 2 * p + s
                        op = 64 * s
                        if not is_width:
                            nc.tensor.matmul(
                                av[op:op + 64, osl],
                                vaug[:, blk * 512 + h * 64: blk * 512 + (h + 1) * 64],
                                aT[:, h * 128:(h + 1) * 128],
                                start=True, stop=True,
                                tile_position=(0, op))
                        else:
                            for yc in range(2):
                                nc.tensor.matmul(
                                    av[op:op + 64, osl],
                                    vaug[:, (2 * blk + yc) * 512 + h * 64:
                                         (2 * blk + yc) * 512 + (h + 1) * 64],
                                    aT[:, yc * 2048 + h * 256:yc * 2048 + (h + 1) * 256],
                                    start=(yc == 0), stop=(yc == 1),
                                    tile_position=(0, op))

                # --- stash unnormalized block (denoms ride at rows 32/96) ---
                bo = (blk % half) * AVW
                nc.vector.tensor_copy(pb[:, bo:bo + AVW], av[:])

            # --- compact denominators, reciprocal, bounce to DRAM ---
            for s in range(2):
                rs = slice(hf * 64 + s * 32, hf * 64 + s * 32 + half)
                nc.sync.dma_start(
                    dn[rs, :],
                    pb[32 + 64 * s: 33 + 64 * s, :].rearrange("p (g f) -> p g f", g=half))
                with nc.allow_low_precision(reason="bf16 softmax denominators"):
                    nc.vector.reciprocal(rec[rs, :], dn[rs, :])
                nc.sync.dma_start(rec_d[rs, :], rec[rs, :])

            # --- normalize per OG-group: batched bcast DMA + one GPSIMD multiply ---
            for g0 in range(hf * half, (hf + 1) * half, OG):
                bc = bc_pool.tile([128, OG * AVW], bf, tag="bc", name="bc")
                for s in range(2):
                    r0 = hf * 64 + s * 32 + (g0 % half)
                    # partition_broadcast yields [o=64, g, f], matching dst order
                    nc.sync.dma_start(
                        bc[64 * s:64 * s + 64, :].rearrange("o (g f) -> o g f", g=OG),
                        rec_d[r0: r0 + OG, :].partition_broadcast(64))
                pbn = pbn_pool.tile([128, 4 * OG * XBLK], bf, tag="pbn", name="pbn")
                bo = (g0 % half) * AVW
                # pbn layout [128, (p:4)(g:OG)(x:XBLK)]; pb/bc are block-major
                nc.gpsimd.tensor_tensor(
                    pbn[:].rearrange("q (p g x) -> q p g x", p=4, g=OG),
                    pb[:, bo:bo + OG * AVW].rearrange("q (g p x) -> q p g x", g=OG, p=4),
                    bc[:].rearrange("q (g p x) -> q p g x", g=OG, p=4), mult)
                for cc in range(2):
                    po = ps_m.tile([128, 512], f32, tag="ps_m", name="ps_m")
                    for p in range(4):
                        nc.tensor.matmul(
                            po[:],
                            wo_sb[p][cc][:],
                            pbn[:, p * 512:(p + 1) * 512],
                            start=(p == 0), stop=(p == 3))
                    osb = osb_pool.tile([128, 512], f32, tag="osb", name="osb")
                    nc.scalar.copy(osb[:], po[:])
                    nc.scalar.dma_start(
                        out_d[cc][:, g0 * XBLK:g0 * XBLK + 512], osb[:])

    with TileContext(nc) as tc:
        with contextlib.ExitStack() as c1:
            phase(tc, c1, xh, wts["wq_h"], wts["wk_h"], wts["wv_h"], wts["wo_h"],
                  out_h, is_width=False)
        with contextlib.ExitStack() as c2:
            phase(tc, c2, xw, wts["wq_w"], wts["wk_w"], wts["wv_w"], wts["wo_w"],
                  out_w, is_width=True)

    nc.compile()
    return nc


def _prep_weights(inp):
    """Host-side weight layouts, bf16."""
    def chunks(Wm):                      # [256, 256] -> [2, 128, 256] (lhsT chunks)
        return np.ascontiguousarray(Wm.reshape(2, 128, 256)).astype(BF16)

    def v_pad(Wm):                       # -> [2, 128, 8*64]: per-head [Wv_h | 0...]
        out = np.zeros((2, 128, 512), np.float32)
        for hh in range(8):
            out[:, :, hh * 64:hh * 64 + 32] = Wm.reshape(2, 128, 256)[:, :, hh * 32:(hh + 1) * 32]
        return out.astype(BF16)

    def wo_aug(Wo):                      # -> [4 pairs, 2 cc, 128 K(padded), 128 M]
        out = np.zeros((4, 2, 128, 128), np.float32)
        for p in range(4):
            for cc in range(2):
                out[p, cc, 0:32, :] = Wo[64 * p:64 * p + 32, cc * 128:(cc + 1) * 128]
                out[p, cc, 64:96, :] = Wo[64 * p + 32:64 * p + 64, cc * 128:(cc + 1) * 128]
        return out.astype(BF16)

    d = {}
    for ph in ("h", "w"):
        d[f"wq_{ph}"] = chunks(np.asarray(inp[f"Wq_{ph}"], np.float32))
        d[f"wk_{ph}"] = chunks(np.asarray(inp[f"Wk_{ph}"], np.float32))
        d[f"wv_{ph}"] = v_pad(np.asarray(inp[f"Wv_{ph}"], np.float32))
        d[f"wo_{ph}"] = wo_aug(np.asarray(inp[f"Wo_{ph}"], np.float32))
    return d


def kernel(x, Wq_h, Wk_h, Wv_h, Wo_h, bo_h, Wq_w, Wk_w, Wv_w, Wo_w, bo_w, h, w,
           _trace=False):
    from concourse.bass_utils import run_bass_kernel_spmd

    x = np.asarray(x, np.float32)
    xs = x.reshape(B, H, W, C)
    wd = _prep_weights(dict(Wq_h=Wq_h, Wk_h=Wk_h, Wv_h=Wv_h, Wo_h=Wo_h,
                            Wq_w=Wq_w, Wk_w=Wk_w, Wv_w=Wv_w, Wo_w=Wo_w))

    in_maps = []
    for core in range(8):
        b, j = divmod(core, 4)
        xh_a = xs[b][:, j * WC:(j + 1) * WC, :].transpose(2, 1, 0)   # [C, Wc, H]
        xw_a = xs[b][j * HC:(j + 1) * HC, :, :].transpose(2, 0, 1)   # [C, Hc, W]
        m = dict(wd)
        m["xh"] = np.ascontiguousarray(xh_a).reshape(2, 128, NTOK).astype(BF16)
        m["xw"] = np.ascontiguousarray(xw_a).reshape(2, 128, NTOK).astype(BF16)
        in_maps.append(m)

    if "nc" not in _compiled:
        _compiled["nc"] = _build_module()
    nc = _compiled["nc"]

    kw = {}
    if _trace:
        kw = dict(trace=True, trace_cores=[0])
    res = run_bass_kernel_spmd(nc, in_maps, core_ids=list(range(8)), **kw)
    _compiled["last_result"] = res

    out = np.zeros((B, H, W, C), np.float32)
    for core in range(8):
        b, j = divmod(core, 4)
        oh = np.asarray(res.results[core]["out_h"], dtype=np.float32)
        ow = np.asarray(res.results[core]["out_w"], dtype=np.float32)
        # outT[c, n], c = cc*128 + ci; height n = w*128 + r -> [r, w, c]
        oh_t = oh.reshape(256, WC, 128).transpose(2, 1, 0)
        out[b, :, j * WC:(j + 1) * WC, :] += oh_t
        # width n = r*256 + wcol -> [r, wcol, c]
        ow_t = ow.reshape(256, HC, 256).transpose(1, 2, 0)
        out[b, j * HC:(j + 1) * HC, :, :] += ow_t
    out += np.asarray(bo_h, np.float32) + np.asarray(bo_w, np.float32)
    return out.reshape(B, H * W, C)


# revision 10
# speedup vs baseline: 1.2566x; 1.0158x over previous
"""AxialAttention Trainium2 kernel: 8-core SPMD, no collectives.

Sharding: core (b, j) computes height-attention for x[b, :, 64j:64j+64, :]
and width-attention for x[b, 32j:32j+32, :, :]; host sums partial outputs.

Bisection variant: baseline scores/exp/AV/PSUM structure + new vaug/pb big
tiles, sync-queue normalize DMAs, and q-copies on ScalarE.
"""

import numpy as np
import ml_dtypes

B, H, W, C = 2, 128, 256, 256
HEADS, D = 8, 32
SCALE = float(D) ** -0.5
WC = W // 4   # 64 w-columns per core (height phase)
HC = H // 4   # 32 h-rows per core (width phase)
NTOK = 8192   # tokens per core per phase
BF16 = ml_dtypes.bfloat16

_compiled = {}


def _build_module():
    import contextlib
    import concourse.bass as bass  # noqa: F401
    from concourse import bacc, mybir
    from concourse.tile import TileContext

    bf = mybir.dt.bfloat16
    f32 = mybir.dt.float32
    Exp = mybir.ActivationFunctionType.Exp
    mult = mybir.AluOpType.mult

    nc = bacc.Bacc("TRN2", target_bir_lowering=False)

    # ---- DRAM I/O ----
    xh = nc.dram_tensor("xh", [2, 128, NTOK], bf, kind="ExternalInput")
    xw = nc.dram_tensor("xw", [2, 128, NTOK], bf, kind="ExternalInput")
    wts = {}
    for ph in ("h", "w"):
        wts[f"wq_{ph}"] = nc.dram_tensor(f"wq_{ph}", [2, 128, 256], bf, kind="ExternalInput")
        wts[f"wk_{ph}"] = nc.dram_tensor(f"wk_{ph}", [2, 128, 256], bf, kind="ExternalInput")
        wts[f"wv_{ph}"] = nc.dram_tensor(f"wv_{ph}", [2, 128, 512], bf, kind="ExternalInput")
        wts[f"wo_{ph}"] = nc.dram_tensor(f"wo_{ph}", [4, 2, 128, 128], bf, kind="ExternalInput")
    out_h = nc.dram_tensor("out_h", [2, 128, WC * 128], bf, kind="ExternalOutput")
    out_w = nc.dram_tensor("out_w", [2, 128, HC * 256], bf, kind="ExternalOutput")

    def phase(tc, ctx, xT_dram, wq_d, wk_d, wv_d, wo_d, out_d, is_width):
        tag = "w" if is_width else "h"
        XBLK = 256 if is_width else 128          # attention span per block
        nblk = HC if is_width else WC            # 32 or 64 blocks
        half = nblk // 2
        AVW = 4 * XBLK                           # av psum width: 4 pairs
        NT = NTOK // 128                         # 64 token tiles
        GRP = 2 if is_width else 4               # blocks per qk projection group
        OG = 2 if is_width else 4                # blocks per oproj group

        pool = ctx.enter_context(tc.tile_pool(name="persist", bufs=1))
        work = ctx.enter_context(tc.tile_pool(name="work", bufs=2))
        at_pool = ctx.enter_context(tc.tile_pool(name="at", bufs=3 if not is_width else 2))
        bc_pool = ctx.enter_context(tc.tile_pool(name="bc", bufs=3))
        pbn_pool = ctx.enter_context(tc.tile_pool(name="pbn", bufs=2))
        osb_pool = ctx.enter_context(tc.tile_pool(name="osb", bufs=2))
        ps_s = ctx.enter_context(tc.tile_pool(name="ps_s", bufs=1, space="PSUM"))
        ps_av = ctx.enter_context(tc.tile_pool(name="ps_av", bufs=2 if not is_width else 1, space="PSUM"))
        ps_m = ctx.enter_context(tc.tile_pool(name="ps_m", bufs=2, space="PSUM"))

        # ---- weights + xT to SBUF ----
        def load(dram_ap, shape, nm):
            t = pool.tile(shape, bf, tag=nm, name=nm)
            nc.sync.dma_start(t[:], dram_ap)
            return t

        wq_sb = [load(wq_d[cc], [128, 256], f"wq{cc}") for cc in range(2)]
        wk_sb = [load(wk_d[cc], [128, 256], f"wk{cc}") for cc in range(2)]
        wv_sb = [load(wv_d[cc], [128, 512], f"wv{cc}") for cc in range(2)]
        wo_sb = [[load(wo_d[p, cc], [128, 128], f"wo{p}_{cc}") for cc in range(2)]
                 for p in range(4)]
        xT = [load(xT_dram[cc], [128, NTOK], f"xT{cc}") for cc in range(2)]

        # ---- rolling v_aug pool: per head [v_h | 1 | 0] (zeros from padded Wv) ----
        vpool = ctx.enter_context(tc.tile_pool(name="vpool", bufs=8))

        def make_vaug(t_i):
            ps = ps_m.tile([128, 512], f32, tag="ps_m", name="ps_m")
            for cc in range(2):
                nc.tensor.matmul(
                    ps[:], xT[cc][:, t_i * 128:(t_i + 1) * 128], wv_sb[cc][:],
                    start=(cc == 0), stop=(cc == 1))
            vt = vpool.tile([128, 512], bf, tag="vaug", name="vaug")
            nc.vector.tensor_copy(vt[:], ps[:])
            nc.vector.memset(vt[:].rearrange("p (h t) -> p h t", t=64)[:, :, 32], 1.0)
            return vt

        # ---- attention main loops ----
        # dn row = hf*64 + s*32 + (blk % half)  (32-aligned (hf,s) groups)
        dn = pool.tile([128, AVW], bf, tag="dn", name="dn")          # compacted denominators
        rec = pool.tile([128, AVW], bf, tag="rec", name="rec")       # their reciprocals
        rec_d = nc.dram_tensor(f"rec_dram_{tag}", [128, AVW], bf)
        nc.vector.memset(dn[:], 1.0)   # width pads stay finite for the recip
        # unnormalized stash, double-buffered so half hf+1 overlaps hf's normalize
        pbs = [pool.tile([128, half * AVW], bf, tag=f"pb{i}", name=f"pb{i}")
               for i in range(2)]

        for hf in range(2):
            pb = pbs[hf]
            blks = list(range(hf * half, (hf + 1) * half))
            qk_groups = {}
            for blk in blks:
                # --- grouped q/k projection over GRP blocks (512 tokens) ---
                g = blk // GRP
                if g not in qk_groups:
                    gtiles = []
                    for ti, w_sb in ((0, wq_sb), (1, wk_sb)):
                        gt = work.tile([128, 1024], bf, tag=f"qkg{ti}", name=f"qkg{ti}")
                        for ic in range(2):
                            psqk = ps_m.tile([128, 512], f32, tag="ps_m", name="ps_m")
                            for cc in range(2):
                                nc.tensor.matmul(
                                    psqk[:],
                                    w_sb[cc][:, ic * 128:(ic + 1) * 128],
                                    xT[cc][:, g * 512:(g + 1) * 512],
                                    start=(cc == 0), stop=(cc == 1))
                            nc.vector.tensor_copy(gt[:, ic * 512:(ic + 1) * 512], psqk[:])
                        gtiles.append(gt)
                    qk_groups = {g: gtiles}      # keep only current group
                qg, kg = qk_groups[g]
                boff = (blk % GRP) * 128 if not is_width else (blk % GRP) * 256
                vts = ([make_vaug(blk)] if not is_width
                       else [make_vaug(2 * blk), make_vaug(2 * blk + 1)])

                if not is_width:
                    aT = at_pool.tile([128, 8 * 128], bf, tag="aT", name="aT")
                    ps = ps_s.tile([128, 2048], f32, tag="s_ps", name="s_ps")
                    for h in range(8):
                        th, hh = divmod(h, 4)
                        col = 512 * hh + 128 * th        # bank = row-group
                        nc.tensor.matmul(
                            ps[:, col:col + 128],
                            kg[hh * 32:(hh + 1) * 32, th * 512 + boff: th * 512 + boff + 128],
                            qg[hh * 32:(hh + 1) * 32, th * 512 + boff: th * 512 + boff + 128],
                            start=True, stop=True,
                            tile_position=(hh * 32, 0))
                    # aT col for head h=4*th+hh is 128*h = 512*th + 128*hh
                    nc.scalar.activation(
                        aT[:].rearrange("p (th hh x) -> p hh th x", th=2, hh=4),
                        ps[:].rearrange("p (hh b) -> p hh b", hh=4)[:, :, 0:256]
                             .rearrange("p hh (th x) -> p hh th x", th=2),
                        Exp, scale=SCALE)
                else:
                    aT = at_pool.tile([128, 2 * 8 * 256], bf, tag="aT", name="aT")
                    for yc in range(2):
                        ps = ps_s.tile([128, 2048], f32, tag="s_ps", name="s_ps")
                        for h in range(8):
                            th, hh = divmod(h, 4)
                            col = 512 * hh + 256 * th    # bank = row-group
                            nc.tensor.matmul(
                                ps[:, col:col + 256],
                                kg[hh * 32:(hh + 1) * 32, th * 512 + boff + yc * 128: th * 512 + boff + (yc + 1) * 128],
                                qg[hh * 32:(hh + 1) * 32, th * 512 + boff: th * 512 + boff + 256],
                                start=True, stop=True,
                                tile_position=(hh * 32, 0))
                        nc.scalar.activation(
                            aT[:, yc * 2048:(yc + 1) * 2048].rearrange(
                                "p (th hh x) -> p hh th x", th=2, hh=4),
                            ps[:].rearrange("p (hh b) -> p hh b", hh=4)[:, :, 0:512]
                                 .rearrange("p hh (th x) -> p hh th x", th=2),
                            Exp, scale=SCALE)

                # --- AV with denominator column, 2-head col packing per pair ---
                av = ps_av.tile([128, AVW], f32, tag="av_ps", name="av_ps")
                for p in range(4):
                    osl = slice(p * XBLK, (p + 1) * XBLK)
                    for s in range(2):
                        h = 2 * p + s
                        op = 64 * s
                        if not is_width:
                            nc.tensor.matmul(
                                av[op:op + 64, osl],
                                vts[0][:, h * 64:(h + 1) * 64],
                                aT[:, h * 128:(h + 1) * 128],
                                start=True, stop=True,
                                tile_position=(0, op))
                        else:
                            for yc in range(2):
                                nc.tensor.matmul(
                                    av[op:op + 64, osl],
                                    vts[yc][:, h * 64:(h + 1) * 64],
                                    aT[:, yc * 2048 + h * 256:yc * 2048 + (h + 1) * 256],
                                    start=(yc == 0), stop=(yc == 1),
                                    tile_position=(0, op))

                # --- stash unnormalized block (denoms ride at rows 32/96) ---
                bo = (blk % half) * AVW
                nc.vector.tensor_copy(pb[:, bo:bo + AVW], av[:])

            # --- compact denominators, one reciprocal per half, bounce to DRAM ---
            for s in range(2):
                rs = slice(hf * 64 + s * 32, hf * 64 + s * 32 + half)
                nc.sync.dma_start(
                    dn[rs, :],
                    pb[32 + 64 * s: 33 + 64 * s, :].rearrange("p (g f) -> p g f", g=half))
            rh = slice(hf * 64, hf * 64 + 64)
            with nc.allow_low_precision(reason="bf16 softmax denominators"):
                nc.vector.reciprocal(rec[rh, :], dn[rh, :])
            nc.sync.dma_start(rec_d[rh, :], rec[rh, :])

            # --- normalize per OG-group: batched bcast DMA + one GPSIMD multiply ---
            for g0 in range(hf * half, (hf + 1) * half, OG):
                bc = bc_pool.tile([128, OG * AVW], bf, tag="bc", name="bc")
                for s in range(2):
                    r0 = hf * 64 + s * 32 + (g0 % half)
                    # partition_broadcast yields [o=64, g, f], matching dst order
                    nc.sync.dma_start(
                        bc[64 * s:64 * s + 64, :].rearrange("o (g f) -> o g f", g=OG),
                        rec_d[r0: r0 + OG, :].partition_broadcast(64))
                pbn = pbn_pool.tile([128, 4 * OG * XBLK], bf, tag="pbn", name="pbn")
                bo = (g0 % half) * AVW
                # pbn layout [128, (p:4)(g:OG)(x:XBLK)]; pb/bc are block-major
                # normalize multiply split: DVE handles every 3rd group (it is
                # ~4x faster per element here, but has less slack than GPSIMD)
                tt_eng = nc.vector if ((g0 // OG) % 3 == 2) else nc.gpsimd
                tt_eng.tensor_tensor(
                    pbn[:].rearrange("q (p g x) -> q p g x", p=4, g=OG),
                    pb[:, bo:bo + OG * AVW].rearrange("q (g p x) -> q p g x", g=OG, p=4),
                    bc[:].rearrange("q (g p x) -> q p g x", g=OG, p=4), mult)
                for cc in range(2):
                    po = ps_m.tile([128, 512], f32, tag="ps_m", name="ps_m")
                    for p in range(4):
                        nc.tensor.matmul(
                            po[:],
                            wo_sb[p][cc][:],
                            pbn[:, p * 512:(p + 1) * 512],
                            start=(p == 0), stop=(p == 3))
                    osb = osb_pool.tile([128, 512], bf, tag="osb", name="osb")
                    nc.scalar.copy(osb[:], po[:])
                    nc.sync.dma_start(
                        out_d[cc][:, g0 * XBLK:g0 * XBLK + 512], osb[:])

    with TileContext(nc) as tc:
        with contextlib.ExitStack() as c1:
            phase(tc, c1, xh, wts["wq_h"], wts["wk_h"], wts["wv_h"], wts["wo_h"],
                  out_h, is_width=False)
        with contextlib.ExitStack() as c2:
            phase(tc, c2, xw, wts["wq_w"], wts["wk_w"], wts["wv_w"], wts["wo_w"],
                  out_w, is_width=True)

    nc.compile()
    return nc


def _prep_weights(inp):
    """Host-side weight layouts, bf16."""
    def chunks(Wm):                      # [256, 256] -> [2, 128, 256] (lhsT chunks)
        return np.ascontiguousarray(Wm.reshape(2, 128, 256)).astype(BF16)

    def v_pad(Wm):                       # -> [2, 128, 8*64]: per-head [Wv_h | 0...]
        out = np.zeros((2, 128, 512), np.float32)
        for hh in range(8):
            out[:, :, hh * 64:hh * 64 + 32] = Wm.reshape(2, 128, 256)[:, :, hh * 32:(hh + 1) * 32]
        return out.astype(BF16)

    def wo_aug(Wo):                      # -> [4 pairs, 2 cc, 128 K(padded), 128 M]
        out = np.zeros((4, 2, 128, 128), np.float32)
        for p in range(4):
            for cc in range(2):
                out[p, cc, 0:32, :] = Wo[64 * p:64 * p + 32, cc * 128:(cc + 1) * 128]
                out[p, cc, 64:96, :] = Wo[64 * p + 32:64 * p + 64, cc * 128:(cc + 1) * 128]
        return out.astype(BF16)

    d = {}
    for ph in ("h", "w"):
        d[f"wq_{ph}"] = chunks(np.asarray(inp[f"Wq_{ph}"], np.float32))
        d[f"wk_{ph}"] = chunks(np.asarray(inp[f"Wk_{ph}"], np.float32))
        d[f"wv_{ph}"] = v_pad(np.asarray(inp[f"Wv_{ph}"], np.float32))
        d[f"wo_{ph}"] = wo_aug(np.asarray(inp[f"Wo_{ph}"], np.float32))
    return d


def kernel(x, Wq_h, Wk_h, Wv_h, Wo_h, bo_h, Wq_w, Wk_w, Wv_w, Wo_w, bo_w, h, w,
           _trace=False):
    from concourse.bass_utils import run_bass_kernel_spmd

    x = np.asarray(x, np.float32)
    xs = x.reshape(B, H, W, C)
    wd = _prep_weights(dict(Wq_h=Wq_h, Wk_h=Wk_h, Wv_h=Wv_h, Wo_h=Wo_h,
                            Wq_w=Wq_w, Wk_w=Wk_w, Wv_w=Wv_w, Wo_w=Wo_w))

    in_maps = []
    for core in range(8):
        b, j = divmod(core, 4)
        xh_a = xs[b][:, j * WC:(j + 1) * WC, :].transpose(2, 1, 0)   # [C, Wc, H]
        xw_a = xs[b][j * HC:(j + 1) * HC, :, :].transpose(2, 0, 1)   # [C, Hc, W]
        m = dict(wd)
        m["xh"] = np.ascontiguousarray(xh_a).reshape(2, 128, NTOK).astype(BF16)
        m["xw"] = np.ascontiguousarray(xw_a).reshape(2, 128, NTOK).astype(BF16)
        in_maps.append(m)

    if "nc" not in _compiled:
        _compiled["nc"] = _build_module()
    nc = _compiled["nc"]

    kw = {}
    if _trace:
        kw = dict(trace=True, trace_cores=[0])
    res = run_bass_kernel_spmd(nc, in_maps, core_ids=list(range(8)), **kw)
    _compiled["last_result"] = res

    out = np.zeros((B, H, W, C), np.float32)
    for core in range(8):
        b, j = divmod(core, 4)
        oh = np.asarray(res.results[core]["out_h"], dtype=np.float32)
        ow = np.asarray(res.results[core]["out_w"], dtype=np.float32)
        # outT[c, n], c = cc*128 + ci; height n = w*128 + r -> [r, w, c]
        oh_t = oh.reshape(256, WC, 128).transpose(2, 1, 0)
        out[b, :, j * WC:(j + 1) * WC, :] += oh_t
        # width n = r*256 + wcol -> [r, wcol, c]
        ow_t = ow.reshape(256, HC, 256).transpose(1, 2, 0)
        out[b, j * HC:(j + 1) * HC, :, :] += ow_t
    out += np.asarray(bo_h, np.float32) + np.asarray(bo_w, np.float32)
    return out.reshape(B, H * W, C)


# revision 12
# speedup vs baseline: 1.6165x; 1.2864x over previous
"""AxialAttention Trainium2 kernel: 8-core SPMD, no collectives.

Sharding: core (b, j) computes height-attention for x[b, :, 64j:64j+64, :]
and width-attention for x[b, 32j:32j+32, :, :]; host sums partial outputs.

Bisection variant: baseline scores/exp/AV/PSUM structure + new vaug/pb big
tiles, sync-queue normalize DMAs, and q-copies on ScalarE.
"""

import numpy as np
import ml_dtypes

B, H, W, C = 2, 128, 256, 256
HEADS, D = 8, 32
SCALE = float(D) ** -0.5
WC = W // 4   # 64 w-columns per core (height phase)
HC = H // 4   # 32 h-rows per core (width phase)
NTOK = 8192   # tokens per core per phase
BF16 = ml_dtypes.bfloat16

_compiled = {}


def _build_module():
    import contextlib
    import concourse.bass as bass  # noqa: F401
    from concourse import bacc, mybir
    from concourse.tile import TileContext

    bf = mybir.dt.bfloat16
    f32 = mybir.dt.float32
    Exp = mybir.ActivationFunctionType.Exp
    mult = mybir.AluOpType.mult

    nc = bacc.Bacc("TRN2", target_bir_lowering=False)

    # ---- DRAM I/O ----
    xh = nc.dram_tensor("xh", [2, 128, NTOK], bf, kind="ExternalInput")
    xw = nc.dram_tensor("xw", [2, 128, NTOK], bf, kind="ExternalInput")
    wts = {}
    for ph in ("h", "w"):
        wts[f"wq_{ph}"] = nc.dram_tensor(f"wq_{ph}", [2, 128, 256], bf, kind="ExternalInput")
        wts[f"wk_{ph}"] = nc.dram_tensor(f"wk_{ph}", [2, 128, 256], bf, kind="ExternalInput")
        wts[f"wv_{ph}"] = nc.dram_tensor(f"wv_{ph}", [2, 128, 512], bf, kind="ExternalInput")
        wts[f"wo_{ph}"] = nc.dram_tensor(f"wo_{ph}", [4, 2, 128, 128], bf, kind="ExternalInput")
    out_h = nc.dram_tensor("out_h", [2, 128, WC * 128], bf, kind="ExternalOutput")
    out_w = nc.dram_tensor("out_w", [2, 128, HC * 256], bf, kind="ExternalOutput")

    def phase(tc, ctx, xT_dram, wq_d, wk_d, wv_d, wo_d, out_d, is_width):
        tag = "w" if is_width else "h"
        XBLK = 256 if is_width else 128          # attention span per block
        nblk = HC if is_width else WC            # 32 or 64 blocks
        half = nblk // 2
        AVW = 4 * XBLK                           # av psum width: 4 pairs
        NT = NTOK // 128                         # 64 token tiles
        GRP = 2 if is_width else 4               # blocks per qk projection group
        OG = 2 if is_width else 4                # blocks per oproj group

        pool = ctx.enter_context(tc.tile_pool(name="persist", bufs=1))
        work = ctx.enter_context(tc.tile_pool(name="work", bufs=2))
        at_pool = ctx.enter_context(tc.tile_pool(name="at", bufs=3 if not is_width else 2))
        bc_pool = ctx.enter_context(tc.tile_pool(name="bc", bufs=3))
        pbn_pool = ctx.enter_context(tc.tile_pool(name="pbn", bufs=2))
        osb_pool = ctx.enter_context(tc.tile_pool(name="osb", bufs=2))
        ps_s = ctx.enter_context(tc.tile_pool(name="ps_s", bufs=1, space="PSUM"))
        ps_av = ctx.enter_context(tc.tile_pool(name="ps_av", bufs=2 if not is_width else 1, space="PSUM"))
        ps_m = ctx.enter_context(tc.tile_pool(name="ps_m", bufs=2, space="PSUM"))

        # ---- weights + xT to SBUF ----
        def load(dram_ap, shape, nm):
            t = pool.tile(shape, bf, tag=nm, name=nm)
            nc.sync.dma_start(t[:], dram_ap)
            return t

        wq_sb = [load(wq_d[cc], [128, 256], f"wq{cc}") for cc in range(2)]
        wk_sb = [load(wk_d[cc], [128, 256], f"wk{cc}") for cc in range(2)]
        wv_sb = [load(wv_d[cc], [128, 512], f"wv{cc}") for cc in range(2)]
        wo_sb = [[load(wo_d[p, cc], [128, 128], f"wo{p}_{cc}") for cc in range(2)]
                 for p in range(4)]
        xT = [load(xT_dram[cc], [128, NTOK], f"xT{cc}") for cc in range(2)]

        # ---- rolling v_aug pool: per head [v_h | 1 | 0] (zeros from padded Wv) ----
        vpool = ctx.enter_context(tc.tile_pool(name="vpool", bufs=8))

        def make_vaug(t_i):
            ps = ps_m.tile([128, 512], f32, tag="ps_m", name="ps_m")
            for cc in range(2):
                nc.tensor.matmul(
                    ps[:], xT[cc][:, t_i * 128:(t_i + 1) * 128], wv_sb[cc][:],
                    start=(cc == 0), stop=(cc == 1))
            vt = vpool.tile([128, 512], bf, tag="vaug", name="vaug")
            nc.vector.tensor_copy(vt[:], ps[:])
            nc.vector.memset(vt[:].rearrange("p (h t) -> p h t", t=64)[:, :, 32], 1.0)
            return vt

        # ---- attention main loops ----
        # dn row = hf*64 + s*32 + (blk % half)  (32-aligned (hf,s) groups)
        dn = pool.tile([128, AVW], bf, tag="dn", name="dn")          # compacted denominators
        rec = pool.tile([128, AVW], bf, tag="rec", name="rec")       # their reciprocals
        rec_d = nc.dram_tensor(f"rec_dram_{tag}", [128, AVW], bf)
        nc.vector.memset(dn[:], 1.0)   # width pads stay finite for the recip
        # unnormalized stash, double-buffered so half hf+1 overlaps hf's normalize
        pbs = [pool.tile([128, half * AVW], bf, tag=f"pb{i}", name=f"pb{i}")
               for i in range(2)]

        # --- emitters -------------------------------------------------
        def emit_compute(blk, hf, qk_groups):
            pb = pbs[hf]
            if True:
                # --- grouped q/k projection over GRP blocks (512 tokens) ---
                g = blk // GRP
                if g not in qk_groups:
                    gtiles = []
                    for ti, w_sb in ((0, wq_sb), (1, wk_sb)):
                        gt = work.tile([128, 1024], bf, tag=f"qkg{ti}", name=f"qkg{ti}")
                        for ic in range(2):
                            psqk = ps_m.tile([128, 512], f32, tag="ps_m", name="ps_m")
                            for cc in range(2):
                                nc.tensor.matmul(
                                    psqk[:],
                                    w_sb[cc][:, ic * 128:(ic + 1) * 128],
                                    xT[cc][:, g * 512:(g + 1) * 512],
                                    start=(cc == 0), stop=(cc == 1))
                            nc.vector.tensor_copy(gt[:, ic * 512:(ic + 1) * 512], psqk[:])
                        gtiles.append(gt)
                    qk_groups.clear()            # keep only current group
                    qk_groups[g] = gtiles
                qg, kg = qk_groups[g]
                boff = (blk % GRP) * 128 if not is_width else (blk % GRP) * 256
                vts = ([make_vaug(blk)] if not is_width
                       else [make_vaug(2 * blk), make_vaug(2 * blk + 1)])

                if not is_width:
                    aT = at_pool.tile([128, 8 * 128], bf, tag="aT", name="aT")
                    ps = ps_s.tile([128, 2048], f32, tag="s_ps", name="s_ps")
                    for h in range(8):
                        th, hh = divmod(h, 4)
                        col = 512 * hh + 128 * th        # bank = row-group
                        nc.tensor.matmul(
                            ps[:, col:col + 128],
                            kg[hh * 32:(hh + 1) * 32, th * 512 + boff: th * 512 + boff + 128],
                            qg[hh * 32:(hh + 1) * 32, th * 512 + boff: th * 512 + boff + 128],
                            start=True, stop=True,
                            tile_position=(hh * 32, 0))
                    # aT col for head h=4*th+hh is 128*h = 512*th + 128*hh
                    nc.scalar.activation(
                        aT[:].rearrange("p (th hh x) -> p hh th x", th=2, hh=4),
                        ps[:].rearrange("p (hh b) -> p hh b", hh=4)[:, :, 0:256]
                             .rearrange("p hh (th x) -> p hh th x", th=2),
                        Exp, scale=SCALE)
                else:
                    aT = at_pool.tile([128, 2 * 8 * 256], bf, tag="aT", name="aT")
                    for yc in range(2):
                        ps = ps_s.tile([128, 2048], f32, tag="s_ps", name="s_ps")
                        for h in range(8):
                            th, hh = divmod(h, 4)
                            col = 512 * hh + 256 * th    # bank = row-group
                            nc.tensor.matmul(
                                ps[:, col:col + 256],
                                kg[hh * 32:(hh + 1) * 32, th * 512 + boff + yc * 128: th * 512 + boff + (yc + 1) * 128],
                                qg[hh * 32:(hh + 1) * 32, th * 512 + boff: th * 512 + boff + 256],
                                start=True, stop=True,
                                tile_position=(hh * 32, 0))
                        nc.scalar.activation(
                            aT[:, yc * 2048:(yc + 1) * 2048].rearrange(
                                "p (th hh x) -> p hh th x", th=2, hh=4),
                            ps[:].rearrange("p (hh b) -> p hh b", hh=4)[:, :, 0:512]
                                 .rearrange("p hh (th x) -> p hh th x", th=2),
                            Exp, scale=SCALE)

                # --- AV with denominator column, 2-head col packing per pair ---
                av = ps_av.tile([128, AVW], f32, tag="av_ps", name="av_ps")
                for p in range(4):
                    osl = slice(p * XBLK, (p + 1) * XBLK)
                    for s in range(2):
                        h = 2 * p + s
                        op = 64 * s
                        if not is_width:
                            nc.tensor.matmul(
                                av[op:op + 64, osl],
                                vts[0][:, h * 64:(h + 1) * 64],
                                aT[:, h * 128:(h + 1) * 128],
                                start=True, stop=True,
                                tile_position=(0, op))
                        else:
                            for yc in range(2):
                                nc.tensor.matmul(
                                    av[op:op + 64, osl],
                                    vts[yc][:, h * 64:(h + 1) * 64],
                                    aT[:, yc * 2048 + h * 256:yc * 2048 + (h + 1) * 256],
                                    start=(yc == 0), stop=(yc == 1),
                                    tile_position=(0, op))

                # --- stash unnormalized block (denoms ride at rows 32/96) ---
                bo = (blk % half) * AVW
                nc.vector.tensor_copy(pb[:, bo:bo + AVW], av[:])

        def emit_half_recip(hf):
            # compact denominators, one reciprocal per half, bounce to DRAM
            pb = pbs[hf]
            for s in range(2):
                rs = slice(hf * 64 + s * 32, hf * 64 + s * 32 + half)
                nc.sync.dma_start(
                    dn[rs, :],
                    pb[32 + 64 * s: 33 + 64 * s, :].rearrange("p (g f) -> p g f", g=half))
            rh = slice(hf * 64, hf * 64 + 64)
            with nc.allow_low_precision(reason="bf16 softmax denominators"):
                nc.vector.reciprocal(rec[rh, :], dn[rh, :])
            nc.sync.dma_start(rec_d[rh, :], rec[rh, :])

        def emit_group_tt(g0, hf):
            # batched reciprocal-broadcast DMA + one normalize multiply
            pb = pbs[hf]
            bc = bc_pool.tile([128, OG * AVW], bf, tag="bc", name="bc")
            for s in range(2):
                r0 = hf * 64 + s * 32 + (g0 % half)
                # partition_broadcast yields [o=64, g, f], matching dst order
                nc.sync.dma_start(
                    bc[64 * s:64 * s + 64, :].rearrange("o (g f) -> o g f", g=OG),
                    rec_d[r0: r0 + OG, :].partition_broadcast(64))
            pbn = pbn_pool.tile([128, 4 * OG * XBLK], bf, tag="pbn", name="pbn")
            bo = (g0 % half) * AVW
            # pbn layout [128, (p:4)(g:OG)(x:XBLK)]; pb/bc are block-major.
            # Alternate the multiply between GPSIMD and DVE (DVE ~3x faster).
            tt_eng = nc.vector if ((g0 // OG) % 2 == 1) else nc.gpsimd
            tt_eng.tensor_tensor(
                pbn[:].rearrange("q (p g x) -> q p g x", p=4, g=OG),
                pb[:, bo:bo + OG * AVW].rearrange("q (g p x) -> q p g x", g=OG, p=4),
                bc[:].rearrange("q (g p x) -> q p g x", g=OG, p=4), mult)
            return pbn

        def emit_group_oproj(g0, pbn):
            for cc in range(2):
                po = ps_m.tile([128, 512], f32, tag="ps_m", name="ps_m")
                for p in range(4):
                    nc.tensor.matmul(
                        po[:],
                        wo_sb[p][cc][:],
                        pbn[:, p * 512:(p + 1) * 512],
                        start=(p == 0), stop=(p == 3))
                osb = osb_pool.tile([128, 512], bf, tag="osb", name="osb")
                nc.scalar.copy(osb[:], po[:])
                nc.sync.dma_start(
                    out_d[cc][:, g0 * XBLK:g0 * XBLK + 512], osb[:])

        # --- schedule: interleave half hf-1's normalize into half hf's blocks ---
        from collections import deque
        K1 = half // 8                 # blocks between normalize-group emissions
        pend_tt = deque()              # (g0, hf) awaiting bc+TT emission
        pend_op = deque()              # (g0, pbn) awaiting oproj emission
        for hf in range(2):
            qk_groups = {}
            for i, blk in enumerate(range(hf * half, (hf + 1) * half)):
                emit_compute(blk, hf, qk_groups)
                if i % K1 == 0 and pend_tt:
                    g0p, hfp = pend_tt.popleft()
                    pend_op.append((g0p, emit_group_tt(g0p, hfp)))
                if i % K1 == K1 // 2 and pend_op:
                    emit_group_oproj(*pend_op.popleft())
            emit_half_recip(hf)
            for g0 in range(hf * half, (hf + 1) * half, OG):
                pend_tt.append((g0, hf))
        while pend_tt:                 # tail: last half's normalize + oproj
            g0p, hfp = pend_tt.popleft()
            pend_op.append((g0p, emit_group_tt(g0p, hfp)))
        while pend_op:
            emit_group_oproj(*pend_op.popleft())

    with TileContext(nc) as tc:
        with contextlib.ExitStack() as c1:
            phase(tc, c1, xh, wts["wq_h"], wts["wk_h"], wts["wv_h"], wts["wo_h"],
                  out_h, is_width=False)
        with contextlib.ExitStack() as c2:
            phase(tc, c2, xw, wts["wq_w"], wts["wk_w"], wts["wv_w"], wts["wo_w"],
                  out_w, is_width=True)

    nc.compile()
    return nc


def _prep_weights(inp):
    """Host-side weight layouts, bf16."""
    def chunks(Wm):                      # [256, 256] -> [2, 128, 256] (lhsT chunks)
        return np.ascontiguousarray(Wm.reshape(2, 128, 256)).astype(BF16)

    def v_pad(Wm):                       # -> [2, 128, 8*64]: per-head [Wv_h | 0...]
        out = np.zeros((2, 128, 512), np.float32)
        for hh in range(8):
            out[:, :, hh * 64:hh * 64 + 32] = Wm.reshape(2, 128, 256)[:, :, hh * 32:(hh + 1) * 32]
        return out.astype(BF16)

    def wo_aug(Wo):                      # -> [4 pairs, 2 cc, 128 K(padded), 128 M]
        out = np.zeros((4, 2, 128, 128), np.float32)
        for p in range(4):
            for cc in range(2):
                out[p, cc, 0:32, :] = Wo[64 * p:64 * p + 32, cc * 128:(cc + 1) * 128]
                out[p, cc, 64:96, :] = Wo[64 * p + 32:64 * p + 64, cc * 128:(cc + 1) * 128]
        return out.astype(BF16)

    d = {}
    for ph in ("h", "w"):
        d[f"wq_{ph}"] = chunks(np.asarray(inp[f"Wq_{ph}"], np.float32))
        d[f"wk_{ph}"] = chunks(np.asarray(inp[f"Wk_{ph}"], np.float32))
        d[f"wv_{ph}"] = v_pad(np.asarray(inp[f"Wv_{ph}"], np.float32))
        d[f"wo_{ph}"] = wo_aug(np.asarray(inp[f"Wo_{ph}"], np.float32))
    return d


def kernel(x, Wq_h, Wk_h, Wv_h, Wo_h, bo_h, Wq_w, Wk_w, Wv_w, Wo_w, bo_w, h, w,
           _trace=False):
    from concourse.bass_utils import run_bass_kernel_spmd

    x = np.asarray(x, np.float32)
    xs = x.reshape(B, H, W, C)
    wd = _prep_weights(dict(Wq_h=Wq_h, Wk_h=Wk_h, Wv_h=Wv_h, Wo_h=Wo_h,
                            Wq_w=Wq_w, Wk_w=Wk_w, Wv_w=Wv_w, Wo_w=Wo_w))

    in_maps = []
    for core in range(8):
        b, j = divmod(core, 4)
        xh_a = xs[b][:, j * WC:(j + 1) * WC, :].transpose(2, 1, 0)   # [C, Wc, H]
        xw_a = xs[b][j * HC:(j + 1) * HC, :, :].transpose(2, 0, 1)   # [C, Hc, W]
        m = dict(wd)
        m["xh"] = np.ascontiguousarray(xh_a).reshape(2, 128, NTOK).astype(BF16)
        m["xw"] = np.ascontiguousarray(xw_a).reshape(2, 128, NTOK).astype(BF16)
        in_maps.append(m)

    if "nc" not in _compiled:
        _compiled["nc"] = _build_module()
    nc = _compiled["nc"]

    kw = {}
    if _trace:
        kw = dict(trace=True, trace_cores=[0])
    res = run_bass_kernel_spmd(nc, in_maps, core_ids=list(range(8)), **kw)
    _compiled["last_result"] = res

    out = np.zeros((B, H, W, C), np.float32)
    for core in range(8):
        b, j = divmod(core, 4)
        oh = np.asarray(res.results[core]["out_h"], dtype=np.float32)
        ow = np.asarray(res.results[core]["out_w"], dtype=np.float32)
        # outT[c, n], c = cc*128 + ci; height n = w*128 + r -> [r, w, c]
        oh_t = oh.reshape(256, WC, 128).transpose(2, 1, 0)
        out[b, :, j * WC:(j + 1) * WC, :] += oh_t
        # width n = r*256 + wcol -> [r, wcol, c]
        ow_t = ow.reshape(256, HC, 256).transpose(1, 2, 0)
        out[b, j * HC:(j + 1) * HC, :, :] += ow_t
    out += np.asarray(bo_h, np.float32) + np.asarray(bo_w, np.float32)
    return out.reshape(B, H * W, C)


# revision 13
# speedup vs baseline: 1.8900x; 1.1692x over previous
"""AxialAttention Trainium2 kernel: 8-core SPMD, no collectives.

Sharding: core (b, j) computes height-attention for x[b, :, 64j:64j+64, :]
and width-attention for x[b, 32j:32j+32, :, :]; host sums partial outputs.

Single pool scope for both phases, software-pipelined: each half's softmax
normalization + output projection is interleaved into the NEXT half's (or
next phase's) compute blocks, so the PE never idles on the normalize chain.

Per-block dataflow (all matmuls bf16, fp32 PSUM):
  qT,kT groups  = W.T @ x  (lhsT = W chunks), DVE-evacuated to SBUF bf16
  vaug (rolling pool) = per head [v_h | 1 | 0...]; ones col -> softmax denom
  scores: k-stationary, 4-head row-group packing (K=32), bank per row-group
  aT = exp(scale*sT) on ScalarE (PSUM -> SBUF bf16)
  AV: stationary [v_h|1] (M=64), 2-head col packing -> av fp32 PSUM
      (height: one [128,512] tile; width: two [128,512] tiles, pairs 01/23)
  pb (per-half stash, 2 buffers) <- av, bf16; denominators ride rows 32/96
  denominator rows batch-compacted by half (sync DMA), one DVE reciprocal,
  DRAM bounce + batched partition-broadcast; normalize multiply alternates
  GPSIMD/DVE per group; oproj accumulates zero-padded Wo' -> bf16 out DMA
Host: reassemble, add biases, sum height+width partial outputs.
"""

import numpy as np
import ml_dtypes

B, H, W, C = 2, 128, 256, 256
HEADS, D = 8, 32
SCALE = float(D) ** -0.5
WC = W // 4   # 64 w-columns per core (height phase)
HC = H // 4   # 32 h-rows per core (width phase)
NTOK = 8192   # tokens per core per phase
BF16 = ml_dtypes.bfloat16

_compiled = {}


def _build_module():
    import contextlib
    from collections import deque
    import concourse.bass as bass  # noqa: F401
    from concourse import bacc, mybir
    from concourse.tile import TileContext

    bf = mybir.dt.bfloat16
    f32 = mybir.dt.float32
    Exp = mybir.ActivationFunctionType.Exp
    mult = mybir.AluOpType.mult

    nc = bacc.Bacc("TRN2", target_bir_lowering=False)

    # ---- DRAM I/O ----
    xh = nc.dram_tensor("xh", [2, 128, NTOK], bf, kind="ExternalInput")
    xw = nc.dram_tensor("xw", [2, 128, NTOK], bf, kind="ExternalInput")
    wts = {}
    for ph in ("h", "w"):
        wts[f"wq_{ph}"] = nc.dram_tensor(f"wq_{ph}", [2, 128, 256], bf, kind="ExternalInput")
        wts[f"wk_{ph}"] = nc.dram_tensor(f"wk_{ph}", [2, 128, 256], bf, kind="ExternalInput")
        wts[f"wv_{ph}"] = nc.dram_tensor(f"wv_{ph}", [2, 128, 512], bf, kind="ExternalInput")
        wts[f"wo_{ph}"] = nc.dram_tensor(f"wo_{ph}", [4, 2, 128, 128], bf, kind="ExternalInput")
    out_h = nc.dram_tensor("out_h", [2, 128, WC * 128], bf, kind="ExternalOutput")
    out_w = nc.dram_tensor("out_w", [2, 128, HC * 256], bf, kind="ExternalOutput")

    def build(tc, ctx):
        pool = ctx.enter_context(tc.tile_pool(name="persist", bufs=1))
        vpool = ctx.enter_context(tc.tile_pool(name="vpool", bufs=8))
        work = ctx.enter_context(tc.tile_pool(name="work", bufs=2))
        at_h = ctx.enter_context(tc.tile_pool(name="at_h", bufs=3))
        at_w = ctx.enter_context(tc.tile_pool(name="at_w", bufs=2))
        bc_pool = ctx.enter_context(tc.tile_pool(name="bc", bufs=3))
        pbn_pool = ctx.enter_context(tc.tile_pool(name="pbn", bufs=3))
        osb_pool = ctx.enter_context(tc.tile_pool(name="osb", bufs=3))
        ps_s = ctx.enter_context(tc.tile_pool(name="ps_s", bufs=1, space="PSUM"))
        ps_av = ctx.enter_context(tc.tile_pool(name="ps_av", bufs=2, space="PSUM"))
        ps_m = ctx.enter_context(tc.tile_pool(name="ps_m", bufs=2, space="PSUM"))

        # shared across phases (same shapes; pool tag rotation orders reuse)
        dn = pool.tile([128, 1024], bf, tag="dn", name="dn")
        rec = pool.tile([128, 1024], bf, tag="rec", name="rec")
        nc.vector.memset(dn[:], 1.0)     # keep width's pad rows finite
        pbs = [pool.tile([128, 16384], bf, tag=f"pb{i}", name=f"pb{i}")
               for i in range(2)]

        def emit_compute(blk, hf, P):
            is_width, XBLK, half, AVW, GRP = (
                P["is_width"], P["XBLK"], P["half"], P["AVW"], P["GRP"])
            xT, wq_sb, wk_sb, wv_sb = P["xT"], P["wq_sb"], P["wk_sb"], P["wv_sb"]
            qk_groups = P["qk_groups"]
            pb = pbs[hf]

            # --- grouped q/k projection over GRP blocks (512 tokens) ---
            g = blk // GRP
            if g not in qk_groups:
                gtiles = []
                for ti, w_sb in ((0, wq_sb), (1, wk_sb)):
                    gt = work.tile([128, 1024], bf, tag=f"qkg{ti}", name=f"qkg{ti}")
                    for ic in range(2):
                        psqk = ps_m.tile([128, 512], f32, tag="ps_m", name="ps_m")
                        for cc in range(2):
                            nc.tensor.matmul(
                                psqk[:],
                                w_sb[cc][:, ic * 128:(ic + 1) * 128],
                                xT[cc][:, g * 512:(g + 1) * 512],
                                start=(cc == 0), stop=(cc == 1))
                        nc.vector.tensor_copy(gt[:, ic * 512:(ic + 1) * 512], psqk[:])
                    gtiles.append(gt)
                qk_groups.clear()        # keep only current group
                qk_groups[g] = gtiles
            qg, kg = qk_groups[g]
            boff = (blk % GRP) * XBLK

            def make_vaug(t_i):
                ps = ps_m.tile([128, 512], f32, tag="ps_m", name="ps_m")
                for cc in range(2):
                    nc.tensor.matmul(
                        ps[:], xT[cc][:, t_i * 128:(t_i + 1) * 128], wv_sb[cc][:],
                        start=(cc == 0), stop=(cc == 1))
                vt = vpool.tile([128, 512], bf, tag="vaug", name="vaug")
                nc.vector.tensor_copy(vt[:], ps[:])
                nc.vector.memset(vt[:].rearrange("p (h t) -> p h t", t=64)[:, :, 32], 1.0)
                return vt

            vts = ([make_vaug(blk)] if not is_width
                   else [make_vaug(2 * blk), make_vaug(2 * blk + 1)])

            # --- scores + exp -> aT; PSUM bank = row-group ---
            if not is_width:
                aT = at_h.tile([128, 8 * 128], bf, tag="aT_h", name="aT_h")
                ps = ps_s.tile([128, 2048], f32, tag="s_ps", name="s_ps")
                for h in range(8):
                    th, hh = divmod(h, 4)
                    col = 512 * hh + 128 * th
                    nc.tensor.matmul(
                        ps[:, col:col + 128],
                        kg[hh * 32:(hh + 1) * 32, th * 512 + boff: th * 512 + boff + 128],
                        qg[hh * 32:(hh + 1) * 32, th * 512 + boff: th * 512 + boff + 128],
                        start=True, stop=True,
                        tile_position=(hh * 32, 0))
                # aT col for head h=4*th+hh is 128*h = 512*th + 128*hh
                nc.scalar.activation(
                    aT[:].rearrange("p (th hh x) -> p hh th x", th=2, hh=4),
                    ps[:].rearrange("p (hh b) -> p hh b", hh=4)[:, :, 0:256]
                         .rearrange("p hh (th x) -> p hh th x", th=2),
                    Exp, scale=SCALE)
            else:
                aT = at_w.tile([128, 2 * 8 * 256], bf, tag="aT_w", name="aT_w")
                for yc in range(2):
                    ps = ps_s.tile([128, 2048], f32, tag="s_ps", name="s_ps")
                    for h in range(8):
                        th, hh = divmod(h, 4)
                        col = 512 * hh + 256 * th
                        nc.tensor.matmul(
                            ps[:, col:col + 256],
                            kg[hh * 32:(hh + 1) * 32,
                               th * 512 + boff + yc * 128: th * 512 + boff + (yc + 1) * 128],
                            qg[hh * 32:(hh + 1) * 32, th * 512 + boff: th * 512 + boff + 256],
                            start=True, stop=True,
                            tile_position=(hh * 32, 0))
                    nc.scalar.activation(
                        aT[:, yc * 2048:(yc + 1) * 2048].rearrange(
                            "p (th hh x) -> p hh th x", th=2, hh=4),
                        ps[:].rearrange("p (hh b) -> p hh b", hh=4)[:, :, 0:512]
                             .rearrange("p hh (th x) -> p hh th x", th=2),
                        Exp, scale=SCALE)

            # --- AV (denominator column rides); [128,512] av tiles, 2 pairs each ---
            bo = (blk % half) * AVW
            npairt = AVW // 512          # 1 for height, 2 for width
            for pt in range(npairt):
                av = ps_av.tile([128, 512], f32, tag="av_ps", name="av_ps")
                for pp in range(4 // npairt):
                    p = pt * (4 // npairt) + pp if npairt == 2 else pp
                    osl = slice(pp * XBLK, (pp + 1) * XBLK) if npairt == 2 else \
                        slice(pp * XBLK, (pp + 1) * XBLK)
                    for s in range(2):
                        h = 2 * p + s
                        op = 64 * s
                        if not is_width:
                            nc.tensor.matmul(
                                av[op:op + 64, osl],
                                vts[0][:, h * 64:(h + 1) * 64],
                                aT[:, h * 128:(h + 1) * 128],
                                start=True, stop=True,
                                tile_position=(0, op))
                        else:
                            for yc in range(2):
                                nc.tensor.matmul(
                                    av[op:op + 64, osl],
                                    vts[yc][:, h * 64:(h + 1) * 64],
                                    aT[:, yc * 2048 + h * 256:yc * 2048 + (h + 1) * 256],
                                    start=(yc == 0), stop=(yc == 1),
                                    tile_position=(0, op))
                nc.vector.tensor_copy(pb[:, bo + pt * 512: bo + (pt + 1) * 512], av[:])

        def emit_half_recip(hf, P):
            half, AVW = P["half"], P["AVW"]
            pb = pbs[hf]
            for s in range(2):
                rs = slice(hf * 64 + s * 32, hf * 64 + s * 32 + half)
                nc.sync.dma_start(
                    dn[rs, 0:AVW],
                    pb[32 + 64 * s: 33 + 64 * s, 0:half * AVW]
                        .rearrange("p (g f) -> p g f", g=half))
            rh = slice(hf * 64, hf * 64 + 64)
            with nc.allow_low_precision(reason="bf16 softmax denominators"):
                nc.vector.reciprocal(rec[rh, 0:AVW], dn[rh, 0:AVW])
            nc.sync.dma_start(P["rec_d"][rh, :], rec[rh, 0:AVW])

        def emit_group_tt(g0, hf, P, gidx):
            half, AVW, OG, XBLK = P["half"], P["AVW"], P["OG"], P["XBLK"]
            pb = pbs[hf]
            bc = bc_pool.tile([128, 2048], bf, tag="bc", name="bc")
            for s in range(2):
                r0 = hf * 64 + s * 32 + (g0 % half)
                # partition_broadcast yields [o=64, g, f], matching dst order
                nc.sync.dma_start(
                    bc[64 * s:64 * s + 64, :].rearrange("o (g f) -> o g f", g=OG),
                    P["rec_d"][r0: r0 + OG, :].partition_broadcast(64))
            pbn = pbn_pool.tile([128, 2048], bf, tag="pbn", name="pbn")
            bo = (g0 % half) * AVW
            # pbn layout [128, (p:4)(g:OG)(x:XBLK)]; pb/bc are block-major.
            # Alternate the multiply between GPSIMD and DVE (DVE ~3x faster).
            tt_eng = nc.vector if (gidx % 2 == 1) else nc.gpsimd
            tt_eng.tensor_tensor(
                pbn[:].rearrange("q (p g x) -> q p g x", p=4, g=OG),
                pb[:, bo:bo + OG * AVW].rearrange("q (g p x) -> q p g x", g=OG, p=4),
                bc[:].rearrange("q (g p x) -> q p g x", g=OG, p=4), mult)
            return pbn

        def emit_group_oproj(g0, pbn, P):
            XBLK, wo_sb, out_d = P["XBLK"], P["wo_sb"], P["out_d"]
            for cc in range(2):
                po = ps_m.tile([128, 512], f32, tag="ps_m", name="ps_m")
                for p in range(4):
                    nc.tensor.matmul(
                        po[:],
                        wo_sb[p][cc][:],
                        pbn[:, p * 512:(p + 1) * 512],
                        start=(p == 0), stop=(p == 3))
                osb = osb_pool.tile([128, 512], bf, tag="osb", name="osb")
                nc.scalar.copy(osb[:], po[:])
                nc.sync.dma_start(
                    out_d[cc][:, g0 * XBLK:g0 * XBLK + 512], osb[:])

        # ---- per-phase contexts ----
        def make_phase(tag, xT_dram, out_d, is_width):
            XBLK = 256 if is_width else 128
            nblk = HC if is_width else WC
            P = dict(
                is_width=is_width, XBLK=XBLK, nblk=nblk, half=nblk // 2,
                AVW=4 * XBLK, GRP=2 if is_width else 4, OG=2 if is_width else 4,
                out_d=out_d, qk_groups={},
                rec_d=nc.dram_tensor(f"rec_dram_{tag}", [128, 4 * XBLK], bf),
            )

            def load(dram_ap, shape, nm):
                t = pool.tile(shape, bf, tag=nm, name=nm)
                nc.sync.dma_start(t[:], dram_ap)
                return t

            P["wq_sb"] = [load(wts[f"wq_{tag}"][cc], [128, 256], f"wq_{tag}{cc}") for cc in range(2)]
            P["wk_sb"] = [load(wts[f"wk_{tag}"][cc], [128, 256], f"wk_{tag}{cc}") for cc in range(2)]
            P["wv_sb"] = [load(wts[f"wv_{tag}"][cc], [128, 512], f"wv_{tag}{cc}") for cc in range(2)]
            P["wo_sb"] = [[load(wts[f"wo_{tag}"][p, cc], [128, 128], f"wo_{tag}{p}_{cc}")
                           for cc in range(2)] for p in range(4)]
            P["xT"] = [load(xT_dram[cc], [128, NTOK], f"xT{cc}") for cc in range(2)]
            return P

        # ---- schedule: drain pending normalize groups inside compute blocks ----
        pend_tt = deque()   # (g0, hf, P)
        pend_op = deque()   # (g0, pbn, P)
        gidx = 0

        def drain_tt():
            nonlocal gidx
            if pend_tt:
                g0p, hfp, Pp = pend_tt.popleft()
                pend_op.append((g0p, emit_group_tt(g0p, hfp, Pp, gidx), Pp))
                gidx += 1

        def drain_op():
            if pend_op:
                g0p, pbn, Pp = pend_op.popleft()
                emit_group_oproj(g0p, pbn, Pp)

        for tag, xT_dram, out_d, is_width in (
                ("h", xh, out_h, False), ("w", xw, out_w, True)):
            P = make_phase(tag, xT_dram, out_d, is_width)
            half, OG = P["half"], P["OG"]
            K1 = max(1, half // 8)       # 8 normalize-group slots per half
            for hf in range(2):
                for i, blk in enumerate(range(hf * half, (hf + 1) * half)):
                    emit_compute(blk, hf, P)
                    if i % K1 == 0:
                        drain_tt()
                    if i % K1 == K1 // 2:
                        drain_op()
                emit_half_recip(hf, P)
                for g0 in range(hf * half, (hf + 1) * half, OG):
                    pend_tt.append((g0, hf, P))
        while pend_tt or pend_op:        # tail: width's last half
            drain_tt()
            drain_op()
            drain_op()

    with TileContext(nc) as tc:
        with contextlib.ExitStack() as c1:
            build(tc, c1)

    nc.compile()
    return nc


def _prep_weights(inp):
    """Host-side weight layouts, bf16."""
    def chunks(Wm):                      # [256, 256] -> [2, 128, 256] (lhsT chunks)
        return np.ascontiguousarray(Wm.reshape(2, 128, 256)).astype(BF16)

    def v_pad(Wm):                       # -> [2, 128, 8*64]: per-head [Wv_h | 0...]
        out = np.zeros((2, 128, 512), np.float32)
        for hh in range(8):
            out[:, :, hh * 64:hh * 64 + 32] = Wm.reshape(2, 128, 256)[:, :, hh * 32:(hh + 1) * 32]
        return out.astype(BF16)

    def wo_aug(Wo):                      # -> [4 pairs, 2 cc, 128 K(padded), 128 M]
        out = np.zeros((4, 2, 128, 128), np.float32)
        for p in range(4):
            for cc in range(2):
                out[p, cc, 0:32, :] = Wo[64 * p:64 * p + 32, cc * 128:(cc + 1) * 128]
                out[p, cc, 64:96, :] = Wo[64 * p + 32:64 * p + 64, cc * 128:(cc + 1) * 128]
        return out.astype(BF16)

    d = {}
    for ph in ("h", "w"):
        d[f"wq_{ph}"] = chunks(np.asarray(inp[f"Wq_{ph}"], np.float32))
        d[f"wk_{ph}"] = chunks(np.asarray(inp[f"Wk_{ph}"], np.float32))
        d[f"wv_{ph}"] = v_pad(np.asarray(inp[f"Wv_{ph}"], np.float32))
        d[f"wo_{ph}"] = wo_aug(np.asarray(inp[f"Wo_{ph}"], np.float32))
    return d


def kernel(x, Wq_h, Wk_h, Wv_h, Wo_h, bo_h, Wq_w, Wk_w, Wv_w, Wo_w, bo_w, h, w,
           _trace=False):
    from concourse.bass_utils import run_bass_kernel_spmd

    x = np.asarray(x, np.float32)
    xs = x.reshape(B, H, W, C)
    wd = _prep_weights(dict(Wq_h=Wq_h, Wk_h=Wk_h, Wv_h=Wv_h, Wo_h=Wo_h,
                            Wq_w=Wq_w, Wk_w=Wk_w, Wv_w=Wv_w, Wo_w=Wo_w))

    in_maps = []
    for core in range(8):
        b, j = divmod(core, 4)
        xh_a = xs[b][:, j * WC:(j + 1) * WC, :].transpose(2, 1, 0)   # [C, Wc, H]
        xw_a = xs[b][j * HC:(j + 1) * HC, :, :].transpose(2, 0, 1)   # [C, Hc, W]
        m = dict(wd)
        m["xh"] = np.ascontiguousarray(xh_a).reshape(2, 128, NTOK).astype(BF16)
        m["xw"] = np.ascontiguousarray(xw_a).reshape(2, 128, NTOK).astype(BF16)
        in_maps.append(m)

    if "nc" not in _compiled:
        _compiled["nc"] = _build_module()
    nc = _compiled["nc"]

    kw = {}
    if _trace:
        kw = dict(trace=True, trace_cores=[0])
    res = run_bass_kernel_spmd(nc, in_maps, core_ids=list(range(8)), **kw)
    _compiled["last_result"] = res

    out = np.zeros((B, H, W, C), np.float32)
    for core in range(8):
        b, j = divmod(core, 4)
        oh = np.asarray(res.results[core]["out_h"], dtype=np.float32)
        ow = np.asarray(res.results[core]["out_w"], dtype=np.float32)
        # outT[c, n], c = cc*128 + ci; height n = w*128 + r -> [r, w, c]
        oh_t = oh.reshape(256, WC, 128).transpose(2, 1, 0)
        out[b, :, j * WC:(j + 1) * WC, :] += oh_t
        # width n = r*256 + wcol -> [r, wcol, c]
        ow_t = ow.reshape(256, HC, 256).transpose(1, 2, 0)
        out[b, j * HC:(j + 1) * HC, :, :] += ow_t
    out += np.asarray(bo_h, np.float32) + np.asarray(bo_w, np.float32)
    return out.reshape(B, H * W, C)
